# revision 28
# baseline (speedup 1.0000x reference)
"""ComplexMoELayer TRN2 kernel: top-1-routed expert MLP across 8 NeuronCores.

The reference runs all 8 experts densely, then keeps only the top-1 expert's
output per token (weighted by its softmax prob).  Everything except the
selected expert is multiplied by exactly 0, so only ~2048 (expert, token)
pairs of MLP work exist in total.

Sharding strategy ("routed slots"): the host computes the gating function
(amp/phase -> scores -> argmax/prob; ~0.05% of total FLOPs) in float64 and
uses it to shard: each expert's token list is split into chunks of <= C
tokens; the 16 chunks are placed two per core ("slot 0/1").  The device
program is SPMD-static: every core runs the identical program = 2 slots of
a dense complex MLP (D=512 -> H=2048 -> D) on C token columns with
slot-specific weights; the host decides which expert's weights and which
tokens fill each slot.  Load is perfectly balanced by construction.

Device layout is feature-major [feature, token] (tokens = matmul moving
columns).  bf16 matmuls, fp32 PSUM.  ComplexModReLU between the layers.
Final combine out[tok] = (o + b2) * prob happens on host during unshard
(disjoint token sets; b2 is added on device via ACT bias).
"""

import numpy as np

import concourse.bass as bass
import concourse.mybir as mybir
import concourse.tile as tile
from concourse import bacc
from concourse.bass_utils import run_bass_kernel_spmd

F32 = mybir.dt.float32
BF16 = mybir.dt.bfloat16
AF = mybir.ActivationFunctionType
ALU = mybir.AluOpType

E, D, H = 8, 512, 2048
B, S = 4, 512
NT = B * S
KD = D // 128      # 4 k-tiles over D
KH = H // 128      # 16 m-tiles over H (and k-tiles for L2)
MD = D // 128      # 4 m-tiles over D
GM = 4             # W1 m-groups (each 512 cols = 4 m-tiles)
NSLOT = 2
EPS = 1e-10

_CACHE: dict = {}
LAST_RESULT = None


def _build_nc(C):
    nc = bacc.Bacc("TRN2", target_bir_lowering=False, debug=False)

    w1_d, w2_d, xp_d, bt_d, out_d = [], [], [], [], []
    for s in range(NSLOT):
        # slot0 W1: group-outermost so each 1MB group is contiguous and
        # can be DMA'd separately (paces PE start); the rest p-major so a
        # single whole-tensor DMA has 32KB contiguous per partition row
        if s == 0:
            w1_d.append(nc.dram_tensor(f"w1_{s}", [GM, 128, 2, KD, 512],
                                       BF16, kind="ExternalInput"))
        else:
            w1_d.append(nc.dram_tensor(f"w1_{s}", [128, GM, 2, KD, 512],
                                       BF16, kind="ExternalInput"))
        w2_d.append(nc.dram_tensor(f"w2_{s}", [128, MD, 2, KH, 128], BF16,
                                   kind="ExternalInput"))
        xp_d.append(nc.dram_tensor(f"xp{s}", [128, 2, KD, C], BF16,
                                   kind="ExternalInput"))
        # bias pack: [0:16] b1r, [16:32] b1i, [32:48] mod_b, [48:52] b2r,
        # [52:56] b2i   (col m <-> feature row m*128+p)
        bt_d.append(nc.dram_tensor(f"bt{s}", [128, 56], F32,
                                   kind="ExternalInput"))
        out_d.append(nc.dram_tensor(f"out{s}", [128, 2, MD, C], BF16,
                                    kind="ExternalOutput"))

    with tile.TileContext(nc) as tc:
        import contextlib

        ctx = contextlib.ExitStack()
        with ctx:
            wp = ctx.enter_context(tc.tile_pool(name="wp", bufs=1))
            xpool = ctx.enter_context(tc.tile_pool(name="xpool", bufs=1))
            btp = ctx.enter_context(tc.tile_pool(name="btp", bufs=1))
            hf = ctx.enter_context(tc.tile_pool(name="hf", bufs=1))
            tmp = ctx.enter_context(tc.tile_pool(name="tmp", bufs=1))
            hp = ctx.enter_context(tc.tile_pool(name="hp", bufs=1))
            op = ctx.enter_context(tc.tile_pool(name="op", bufs=1))
            pp = ctx.enter_context(tc.tile_pool(name="pp", bufs=2, space="PSUM"))

            eps_sb = btp.tile([128, 1], F32)
            nc.vector.memset(eps_sb, EPS)

            # ---- DMAs.  sync queue: x, biases, W*r; scalar queue: W*i.
            # Weights are m-group sliced so the PE can start after ~1MB.
            bt_sb, xp_sb, w1_sb, w2_sb = [], [], [], []
            # DMA order == PE consumption order (x, W1 s0, W1 s1, W2 s0,
            # W2 s1); alternate groups across the two HWDGE queues.
            qs = [nc.sync, nc.scalar]
            for s in range(NSLOT):
                bt = btp.tile([128, 56], F32, tag=f"bt{s}")
                nc.scalar.dma_start(out=bt, in_=bt_d[s][:])
                bt_sb.append(bt)
                xp = xpool.tile([128, 2, KD, C], BF16, tag=f"xp{s}")
                nc.sync.dma_start(out=xp, in_=xp_d[s][:])
                xp_sb.append(xp)
            for s in range(NSLOT):
                t1 = wp.tile([128, GM, 2, KD, 512], BF16, tag=f"w1_{s}")
                if s == 0:  # fine pacing for PE start
                    for g in range(GM):
                        qs[g % 2].dma_start(out=t1[:, g], in_=w1_d[s][g])
                else:
                    qs[1].dma_start(out=t1, in_=w1_d[s][:])
                w1_sb.append(t1)
            for s in range(NSLOT):
                t2 = wp.tile([128, MD, 2, KH, 128], BF16, tag=f"w2_{s}")
                qs[s % 2].dma_start(out=t2, in_=w2_d[s][:])
                w2_sb.append(t2)

            hrf_s, hif_s, h_r_s, h_i_s, h_n_s = {}, {}, {}, {}, {}

            def emit_l1(s):
                xr = [xp_sb[s][:, 0, k, :] for k in range(KD)]
                xi = [xp_sb[s][:, 1, k, :] for k in range(KD)]
                xn_t = xpool.tile([128, KD, C], BF16, tag=f"xn{s}")
                nc.vector.tensor_scalar(
                    out=xn_t, in0=xp_sb[s][:, 1], scalar1=-1.0, scalar2=None,
                    op0=ALU.mult)
                xn = [xn_t[:, k, :] for k in range(KD)]
                hrf = hf.tile([128, KH, C], BF16, tag=f"hrf{s}")
                hif = hf.tile([128, KH, C], BF16, tag=f"hif{s}")
                bt = bt_sb[s]
                for m in range(KH):
                    g, j = m // 4, m % 4
                    msl = bass.ts(j, 128)
                    ps_hr = pp.tile([128, C], F32, tag="pa", name=f"pa{s}_{m}",
                                    bufs=3)
                    ps_hi = pp.tile([128, C], F32, tag="pc", name=f"pc{s}_{m}",
                                    bufs=3)
                    for k in range(KD):
                        l1r = w1_sb[s][:, g, 0, k, msl]
                        l1i = w1_sb[s][:, g, 1, k, msl]
                        nc.tensor.matmul(ps_hr, l1r, xr[k],
                                         start=(k == 0), stop=False)
                        nc.tensor.matmul(ps_hi, l1i, xr[k],
                                         start=(k == 0), stop=False)
                        nc.tensor.matmul(ps_hi, l1r, xi[k],
                                         start=False, stop=(k == KD - 1))
                        nc.tensor.matmul(ps_hr, l1i, xn[k],
                                         start=False, stop=(k == KD - 1))
                    nc.scalar.activation(out=hrf[:, m, :], in_=ps_hr,
                                         func=AF.Identity,
                                         bias=bt[:, m:m + 1])
                    nc.vector.tensor_scalar(out=hif[:, m, :], in0=ps_hi,
                                            scalar1=bt[:, 16 + m:17 + m],
                                            scalar2=None, op0=ALU.add)
                hrf_s[s], hif_s[s] = hrf, hif

            def emit_modrelu(s):
                hrf, hif = hrf_s[s], hif_s[s]
                bt = bt_sb[s]
                t1 = tmp.tile([128, KH, C], F32, tag="t1", name=f"t1_{s}")
                nc.vector.tensor_tensor(out=t1, in0=hrf, in1=hrf, op=ALU.mult)
                t2 = tmp.tile([128, KH, C], F32, tag="t2", name=f"t2_{s}")
                nc.scalar.activation(out=t2, in_=hif, func=AF.Square)
                nc.vector.tensor_tensor(out=t1, in0=t1, in1=t2, op=ALU.add)
                nc.scalar.activation(out=t1, in_=t1, func=AF.Sqrt, bias=eps_sb)
                # num = relu(a + mod_b) -> staged in h_r's buffer (bf16)
                h_r = hp.tile([128, KH, C], BF16, tag=f"hr{s}", name=f"num{s}")
                for m in range(KH):  # per-m bias
                    nc.scalar.activation(out=h_r[:, m, :], in_=t1[:, m, :],
                                         func=AF.Relu,
                                         bias=bt[:, 32 + m:33 + m])
                # in-place: every read of a (the relus) is already emitted
                nc.vector.reciprocal_approx_fast(out=t1, in_=t1)
                # scale = num / a  (in-place over num)
                nc.vector.tensor_tensor(out=h_r, in0=h_r, in1=t1, op=ALU.mult)
                h_i = hp.tile([128, KH, C], BF16, tag=f"hi{s}")
                nc.vector.tensor_tensor(out=h_i, in0=hif, in1=h_r, op=ALU.mult)
                # h_r overwrites scale last (h_i already consumed it)
                nc.vector.tensor_tensor(out=h_r, in0=hrf, in1=h_r, op=ALU.mult)
                h_r_s[s], h_i_s[s] = h_r, h_i

            def emit_l2(s):
                h_r, h_i = h_r_s[s], h_i_s[s]
                bt = bt_sb[s]
                o_t = op.tile([128, 2, MD, C], BF16, tag="o", name=f"o_{s}")
                for m4 in range(MD):
                    ps_a = pp.tile([128, C], F32, tag="pa", name=f"qa{s}_{m4}",
                                   bufs=3)
                    ps_b = pp.tile([128, C], F32, tag="pb", name=f"qb{s}_{m4}")
                    ps_c = pp.tile([128, C], F32, tag="pc", name=f"qc{s}_{m4}",
                                   bufs=3)
                    for k in range(KH):
                        l2r = w2_sb[s][:, m4, 0, k, :]
                        l2i = w2_sb[s][:, m4, 1, k, :]
                        nc.tensor.matmul(ps_a, l2r, h_r[:, k, :],
                                         start=(k == 0), stop=(k == KH - 1))
                        nc.tensor.matmul(ps_b, l2i, h_i[:, k, :],
                                         start=(k == 0), stop=(k == KH - 1))
                        nc.tensor.matmul(ps_c, l2i, h_r[:, k, :],
                                         start=(k == 0), stop=False)
                        nc.tensor.matmul(ps_c, l2r, h_i[:, k, :],
                                         start=False, stop=(k == KH - 1))
                    sb_b = tmp.tile([128, C], F32, tag="sbb",
                                    name=f"sbb{s}_{m4}", bufs=2)
                    nc.scalar.copy(out=sb_b, in_=ps_b)
                    nc.vector.scalar_tensor_tensor(
                        out=o_t[:, 0, m4, :], in0=ps_a,
                        scalar=bt[:, 48 + m4:49 + m4],
                        in1=sb_b, op0=ALU.add, op1=ALU.subtract)
                    nc.scalar.activation(out=o_t[:, 1, m4, :], in_=ps_c,
                                         func=AF.Identity,
                                         bias=bt[:, 52 + m4:53 + m4])
                nc.gpsimd.dma_start(out=out_d[s][:], in_=o_t)

            # PE order: s0L1, s1L1, s0L2, s1L2 (no PE bubble while s0's
            # ModReLU chain runs on the vector engines during s1L1).
            emit_l1(0)
            emit_modrelu(0)
            emit_l1(1)
            emit_l2(0)
            emit_modrelu(1)
            emit_l2(1)

    nc.compile()
    return nc


def _pack_weights(W1r, W1i, W2r, W2i, b1r, b1i, modb, b2r, b2i):
    """Per-expert packed device arrays (bf16 weights, f32 bias pack).

    w1g: [GM, 128, 2, KD, 512] (group-outer, for the paced slot-0 load)
    w1p: [128, GM, 2, KD, 512] (p-major, single-DMA)
    w2p: [128, MD, 2, KH, 128] (p-major)
    """
    bf = mybir.dt.np(BF16)
    pw = []
    for e in range(E):
        # [k, p, g, c] -> [g, p, k, c]
        a1r = W1r[e].reshape(KD, 128, GM, 512).transpose(2, 1, 0, 3)
        a1i = W1i[e].reshape(KD, 128, GM, 512).transpose(2, 1, 0, 3)
        w1g = np.ascontiguousarray(
            np.stack([a1r, a1i], axis=2)).astype(bf)       # [g,p,ri,k,c]
        w1p = np.ascontiguousarray(w1g.transpose(1, 0, 2, 3, 4))
        a2r = W2r[e].reshape(KH, 128, MD, 128).transpose(2, 1, 0, 3)
        a2i = W2i[e].reshape(KH, 128, MD, 128).transpose(2, 1, 0, 3)
        w2p = np.ascontiguousarray(
            np.stack([a2r, a2i], axis=2).transpose(1, 0, 2, 3, 4)).astype(bf)
        bt = np.empty((128, 56), np.float32)
        bt[:, 0:16] = b1r[e].reshape(KH, 128).T
        bt[:, 16:32] = b1i[e].reshape(KH, 128).T
        bt[:, 32:48] = modb[e].reshape(KH, 128).T
        bt[:, 48:52] = b2r[e].reshape(MD, 128).T
        bt[:, 52:56] = b2i[e].reshape(MD, 128).T
        pw.append((w1g, w1p, w2p, np.ascontiguousarray(bt)))
    return pw


def _choose_capacity(counts):
    """Smallest C (multiple of 8) with sum_e ceil(c_e/C) <= 16 slots."""
    for c in range(32, 513, 8):
        if sum(-(-n // c) for n in counts if n) <= NSLOT * E:
            return c
    return 512


def kernel(**inputs):
    global LAST_RESULT
    f32 = lambda a: np.ascontiguousarray(np.asarray(a, dtype=np.float32))
    xr = f32(inputs["x_real"]).reshape(NT, D)
    xi = f32(inputs["x_imag"]).reshape(NT, D)
    gW = f32(inputs["gate_W"])
    gb = f32(inputs["gate_b"])

    # ---- gating on host (float64; reference is fp32 -> argmax margins are
    # ~2.5e-4, far above either rounding level)
    xr64, xi64 = xr.astype(np.float64), xi.astype(np.float64)
    amp = np.sqrt(xr64 * xr64 + xi64 * xi64)
    phase = np.arctan2(xi64, xr64)
    scores = amp @ gW[:D].astype(np.float64) + phase @ gW[D:].astype(np.float64)
    scores += gb.astype(np.float64)
    idx = scores.argmax(-1)
    sm = np.exp(scores - scores.max(-1, keepdims=True))
    w_top = np.take_along_axis(sm, idx[:, None], 1)[:, 0] / sm.sum(-1)

    counts = np.bincount(idx, minlength=E)
    C = _choose_capacity(counts)

    # ---- chunks: (expert, token_idx_array), <= C tokens each, 16 total
    chunks = []
    for e in range(E):
        toks = np.nonzero(idx == e)[0]
        for i in range(0, len(toks), C):
            chunks.append((e, toks[i:i + C]))
    while len(chunks) < NSLOT * E:
        chunks.append((0, np.empty(0, np.int64)))
    assert len(chunks) == NSLOT * E, f"capacity search failed: {counts}"

    if ("nc", C) not in _CACHE:
        _CACHE[("nc", C)] = _build_nc(C)
    nc = _CACHE[("nc", C)]

    wkey = (gW.shape, float(inputs["W1r"][0, 0, 0]), float(inputs["W2i"][-1, -1, -1]))
    if _CACHE.get("wkey") != wkey:
        _CACHE["pw"] = _pack_weights(
            f32(inputs["W1r"]), f32(inputs["W1i"]),
            f32(inputs["W2r"]), f32(inputs["W2i"]),
            f32(inputs["b1r"]), f32(inputs["b1i"]), f32(inputs["mod_b"]),
            f32(inputs["b2r"]), f32(inputs["b2i"]))
        _CACHE["wkey"] = wkey
    pw = _CACHE["pw"]

    bf = mybir.dt.np(BF16)
    in_maps = []
    for c in range(E):
        m = {}
        for s in range(NSLOT):
            e, toks = chunks[NSLOT * c + s]
            w1g, w1p, w2p, bt = pw[e]
            m[f"w1_{s}"] = w1g if s == 0 else w1p
            m[f"w2_{s}"] = w2p
            m[f"bt{s}"] = bt
            xp = np.zeros((128, 2, KD, C), bf)
            n = len(toks)
            if n:
                xp[:, 0, :, :n] = xr[toks].T.reshape(KD, 128, n).transpose(1, 0, 2)
                xp[:, 1, :, :n] = xi[toks].T.reshape(KD, 128, n).transpose(1, 0, 2)
            m[f"xp{s}"] = xp
        in_maps.append(m)

    res = run_bass_kernel_spmd(nc, in_maps, list(range(E)))
    LAST_RESULT = res

    out_r = np.zeros((NT, D), np.float32)
    out_i = np.zeros((NT, D), np.float32)
    for c in range(E):
        for s in range(NSLOT):
            e, toks = chunks[NSLOT * c + s]
            n = len(toks)
            if not n:
                continue
            o = np.asarray(res.results[c][f"out{s}"], np.float32)
            wv = w_top[toks].astype(np.float32)[:, None]
            out_r[toks] = o[:, 0].transpose(1, 0, 2).reshape(D, C)[:, :n].T * wv
            out_i[toks] = o[:, 1].transpose(1, 0, 2).reshape(D, C)[:, :n].T * wv
    return out_r.reshape(B, S, D), out_i.reshape(B, S, D)


# revision 31
# speedup vs baseline: 1.1460x; 1.1460x over previous
"""ComplexMoELayer TRN2 kernel: top-1-routed expert MLP across 8 NeuronCores.

The reference runs all 8 experts densely, then keeps only the top-1 expert's
output per token (weighted by its softmax prob).  Everything except the
selected expert is multiplied by exactly 0, so only ~2048 (expert, token)
pairs of MLP work exist in total.

Sharding strategy ("routed slots"): the host computes the gating function
(amp/phase -> scores -> argmax/prob; ~0.05% of total FLOPs) in float64 and
uses it to shard: each expert's token list is split into chunks of <= C
tokens; the 16 chunks are placed two per core ("slot 0/1").  The device
program is SPMD-static: every core runs the identical program = 2 slots of
a dense complex MLP (D=512 -> H=2048 -> D) on C token columns with
slot-specific weights; the host decides which expert's weights and which
tokens fill each slot.  Load is perfectly balanced by construction.

Device layout is feature-major [feature, token] (tokens = matmul moving
columns).  bf16 matmuls, fp32 PSUM.  ComplexModReLU between the layers.
Final combine out[tok] = (o + b2) * prob happens on host during unshard
(disjoint token sets; b2 is added on device via ACT bias).
"""

import numpy as np

import concourse.bass as bass
import concourse.mybir as mybir
import concourse.tile as tile
from concourse import bacc
from concourse.bass_utils import run_bass_kernel_spmd

F32 = mybir.dt.float32
BF16 = mybir.dt.bfloat16
AF = mybir.ActivationFunctionType
ALU = mybir.AluOpType

E, D, H = 8, 512, 2048
B, S = 4, 512
NT = B * S
KD = D // 128      # 4 k-tiles over D
KH = H // 128      # 16 m-tiles over H (and k-tiles for L2)
MD = D // 128      # 4 m-tiles over D
GM = 4             # W1 m-groups (each 512 cols = 4 m-tiles)
NSLOT = 2
EPS = 1e-10

_CACHE: dict = {}
LAST_RESULT = None


def _build_nc(C):
    nc = bacc.Bacc("TRN2", target_bir_lowering=False, debug=False)

    w1_d, w2_d, xp_d, bt_d, out_d = [], [], [], [], []
    for s in range(NSLOT):
        # slot0 W1: group-outermost so each 1MB group is contiguous and
        # can be DMA'd separately (paces PE start); the rest p-major so a
        # single whole-tensor DMA has 32KB contiguous per partition row
        if s == 0:
            w1_d.append(nc.dram_tensor(f"w1_{s}", [GM, 128, 2, KD, 512],
                                       BF16, kind="ExternalInput"))
        else:
            w1_d.append(nc.dram_tensor(f"w1_{s}", [128, GM, 2, KD, 512],
                                       BF16, kind="ExternalInput"))
        w2_d.append(nc.dram_tensor(f"w2_{s}", [128, MD, 2, KH, 128], BF16,
                                   kind="ExternalInput"))
        xp_d.append(nc.dram_tensor(f"xp{s}", [128, 2, KD, C], BF16,
                                   kind="ExternalInput"))
        # bias pack: [0:16] b1r, [16:32] b1i, [32:48] mod_b, [48:52] b2r,
        # [52:56] b2i   (col m <-> feature row m*128+p)
        bt_d.append(nc.dram_tensor(f"bt{s}", [128, 56], F32,
                                   kind="ExternalInput"))
        out_d.append(nc.dram_tensor(f"out{s}", [128, 2, MD, C], BF16,
                                    kind="ExternalOutput"))

    with tile.TileContext(nc) as tc:
        import contextlib

        ctx = contextlib.ExitStack()
        with ctx:
            wp = ctx.enter_context(tc.tile_pool(name="wp", bufs=1))
            xpool = ctx.enter_context(tc.tile_pool(name="xpool", bufs=1))
            btp = ctx.enter_context(tc.tile_pool(name="btp", bufs=1))
            hf = ctx.enter_context(tc.tile_pool(name="hf", bufs=1))
            tmp = ctx.enter_context(tc.tile_pool(name="tmp", bufs=1))
            hp = ctx.enter_context(tc.tile_pool(name="hp", bufs=1))
            op = ctx.enter_context(tc.tile_pool(name="op", bufs=1))
            pp = ctx.enter_context(tc.tile_pool(name="pp", bufs=2, space="PSUM"))

            eps_sb = btp.tile([128, 1], F32)
            nc.vector.memset(eps_sb, EPS)

            # ---- DMAs.  sync queue: x, biases, W*r; scalar queue: W*i.
            # Weights are m-group sliced so the PE can start after ~1MB.
            bt_sb, xp_sb, w1_sb, w2_sb = [], [], [], []
            # sync queue: W1 s0 groups (paced, first bytes PE needs), then
            # W2 s0.  scalar queue: x/bias (small), W1 s1, W2 s1.
            t1_0 = wp.tile([128, GM, 2, KD, 512], BF16, tag="w1_0")
            for g in range(GM):
                nc.sync.dma_start(out=t1_0[:, g], in_=w1_d[0][g])
            for s in range(NSLOT):
                bt = btp.tile([128, 56], F32, tag=f"bt{s}")
                nc.scalar.dma_start(out=bt, in_=bt_d[s][:])
                bt_sb.append(bt)
                xp = xpool.tile([128, 2, KD, C], BF16, tag=f"xp{s}")
                nc.scalar.dma_start(out=xp, in_=xp_d[s][:])
                xp_sb.append(xp)
            t1_1 = wp.tile([128, GM, 2, KD, 512], BF16, tag="w1_1")
            nc.scalar.dma_start(out=t1_1, in_=w1_d[1][:])
            w1_sb = [t1_0, t1_1]
            for s in range(NSLOT):
                t2 = wp.tile([128, MD, 2, KH, 128], BF16, tag=f"w2_{s}")
                (nc.sync if s == 0 else nc.scalar).dma_start(
                    out=t2, in_=w2_d[s][:])
                w2_sb.append(t2)

            hrf_s, hif_s, h_r_s, h_i_s, h_n_s = {}, {}, {}, {}, {}

            def emit_l1(s):
                xr = [xp_sb[s][:, 0, k, :] for k in range(KD)]
                xi = [xp_sb[s][:, 1, k, :] for k in range(KD)]
                xn_t = xpool.tile([128, KD, C], BF16, tag=f"xn{s}")
                nc.vector.tensor_scalar(
                    out=xn_t, in0=xp_sb[s][:, 1], scalar1=-1.0, scalar2=None,
                    op0=ALU.mult)
                xn = [xn_t[:, k, :] for k in range(KD)]
                hrf = hf.tile([128, KH, C], BF16, tag=f"hrf{s}")
                hif = hf.tile([128, KH, C], BF16, tag=f"hif{s}")
                bt = bt_sb[s]
                for m in range(KH):
                    g, j = m // 4, m % 4
                    msl = bass.ts(j, 128)
                    ps_hr = pp.tile([128, C], F32, tag="pa", name=f"pa{s}_{m}",
                                    bufs=3)
                    ps_hi = pp.tile([128, C], F32, tag="pc", name=f"pc{s}_{m}",
                                    bufs=3)
                    for k in range(KD):
                        l1r = w1_sb[s][:, g, 0, k, msl]
                        l1i = w1_sb[s][:, g, 1, k, msl]
                        nc.tensor.matmul(ps_hr, l1r, xr[k],
                                         start=(k == 0), stop=False)
                        nc.tensor.matmul(ps_hi, l1i, xr[k],
                                         start=(k == 0), stop=False)
                        nc.tensor.matmul(ps_hi, l1r, xi[k],
                                         start=False, stop=(k == KD - 1))
                        nc.tensor.matmul(ps_hr, l1i, xn[k],
                                         start=False, stop=(k == KD - 1))
                    nc.scalar.activation(out=hrf[:, m, :], in_=ps_hr,
                                         func=AF.Identity,
                                         bias=bt[:, m:m + 1])
                    nc.scalar.activation(out=hif[:, m, :], in_=ps_hi,
                                         func=AF.Identity,
                                         bias=bt[:, 16 + m:17 + m])
                hrf_s[s], hif_s[s] = hrf, hif

            def emit_modrelu(s):
                hrf, hif = hrf_s[s], hif_s[s]
                bt = bt_sb[s]
                t1 = tmp.tile([128, KH, C], F32, tag="t1", name=f"t1_{s}")
                nc.vector.tensor_tensor(out=t1, in0=hrf, in1=hrf, op=ALU.mult)
                t2 = tmp.tile([128, KH, C], F32, tag="t2", name=f"t2_{s}")
                nc.vector.tensor_tensor(out=t2, in0=hif, in1=hif, op=ALU.mult)
                nc.vector.tensor_tensor(out=t1, in0=t1, in1=t2, op=ALU.add)
                nc.scalar.activation(out=t1, in_=t1, func=AF.Sqrt, bias=eps_sb)
                # num = relu(a + mod_b) -> staged in h_r's buffer (bf16)
                h_r = hp.tile([128, KH, C], BF16, tag=f"hr{s}", name=f"num{s}")
                for m in range(KH):  # per-m bias; DVE two-op tensor_scalar
                    nc.vector.tensor_scalar(out=h_r[:, m, :], in0=t1[:, m, :],
                                            scalar1=bt[:, 32 + m:33 + m],
                                            scalar2=0.0, op0=ALU.add,
                                            op1=ALU.max)
                # in-place: every read of a (the relus) is already emitted
                nc.vector.reciprocal_approx_fast(out=t1, in_=t1)
                # scale = num / a  (in-place over num)
                nc.vector.tensor_tensor(out=h_r, in0=h_r, in1=t1, op=ALU.mult)
                h_i = hp.tile([128, KH, C], BF16, tag=f"hi{s}")
                nc.vector.tensor_tensor(out=h_i, in0=hif, in1=h_r, op=ALU.mult)
                # h_r overwrites scale last (h_i already consumed it)
                nc.vector.tensor_tensor(out=h_r, in0=hrf, in1=h_r, op=ALU.mult)
                h_r_s[s], h_i_s[s] = h_r, h_i

            def emit_l2(s):
                h_r, h_i = h_r_s[s], h_i_s[s]
                bt = bt_sb[s]
                o_t = op.tile([128, 2, MD, C], BF16, tag="o", name=f"o_{s}")
                for m4 in range(MD):
                    ps_a = pp.tile([128, C], F32, tag="pa", name=f"qa{s}_{m4}",
                                   bufs=3)
                    ps_b = pp.tile([128, C], F32, tag="pb", name=f"qb{s}_{m4}")
                    ps_c = pp.tile([128, C], F32, tag="pc", name=f"qc{s}_{m4}",
                                   bufs=3)
                    for k in range(KH):
                        l2r = w2_sb[s][:, m4, 0, k, :]
                        l2i = w2_sb[s][:, m4, 1, k, :]
                        nc.tensor.matmul(ps_a, l2r, h_r[:, k, :],
                                         start=(k == 0), stop=(k == KH - 1))
                        nc.tensor.matmul(ps_b, l2i, h_i[:, k, :],
                                         start=(k == 0), stop=(k == KH - 1))
                        nc.tensor.matmul(ps_c, l2i, h_r[:, k, :],
                                         start=(k == 0), stop=False)
                        nc.tensor.matmul(ps_c, l2r, h_i[:, k, :],
                                         start=False, stop=(k == KH - 1))
                    sb_b = tmp.tile([128, C], F32, tag="sbb",
                                    name=f"sbb{s}_{m4}", bufs=2)
                    nc.scalar.copy(out=sb_b, in_=ps_b)
                    nc.vector.scalar_tensor_tensor(
                        out=o_t[:, 0, m4, :], in0=ps_a,
                        scalar=bt[:, 48 + m4:49 + m4],
                        in1=sb_b, op0=ALU.add, op1=ALU.subtract)
                    nc.scalar.activation(out=o_t[:, 1, m4, :], in_=ps_c,
                                         func=AF.Identity,
                                         bias=bt[:, 52 + m4:53 + m4])
                nc.gpsimd.dma_start(out=out_d[s][:], in_=o_t)

            # PE order: s0L1, s1L1, s0L2, s1L2 (no PE bubble while s0's
            # ModReLU chain runs on the vector engines during s1L1).
            emit_l1(0)
            emit_modrelu(0)
            emit_l1(1)
            emit_l2(0)
            emit_modrelu(1)
            emit_l2(1)

    nc.compile()
    return nc


def _pack_weights(W1r, W1i, W2r, W2i, b1r, b1i, modb, b2r, b2i):
    """Per-expert packed device arrays (bf16 weights, f32 bias pack).

    w1g: [GM, 128, 2, KD, 512] (group-outer, for the paced slot-0 load)
    w1p: [128, GM, 2, KD, 512] (p-major, single-DMA)
    w2p: [128, MD, 2, KH, 128] (p-major)
    """
    bf = mybir.dt.np(BF16)
    pw = []
    for e in range(E):
        # [k, p, g, c] -> [g, p, k, c]
        a1r = W1r[e].reshape(KD, 128, GM, 512).transpose(2, 1, 0, 3)
        a1i = W1i[e].reshape(KD, 128, GM, 512).transpose(2, 1, 0, 3)
        w1g = np.ascontiguousarray(
            np.stack([a1r, a1i], axis=2)).astype(bf)       # [g,p,ri,k,c]
        w1p = np.ascontiguousarray(w1g.transpose(1, 0, 2, 3, 4))
        a2r = W2r[e].reshape(KH, 128, MD, 128).transpose(2, 1, 0, 3)
        a2i = W2i[e].reshape(KH, 128, MD, 128).transpose(2, 1, 0, 3)
        w2p = np.ascontiguousarray(
            np.stack([a2r, a2i], axis=2).transpose(1, 0, 2, 3, 4)).astype(bf)
        bt = np.empty((128, 56), np.float32)
        bt[:, 0:16] = b1r[e].reshape(KH, 128).T
        bt[:, 16:32] = b1i[e].reshape(KH, 128).T
        bt[:, 32:48] = modb[e].reshape(KH, 128).T
        bt[:, 48:52] = b2r[e].reshape(MD, 128).T
        bt[:, 52:56] = b2i[e].reshape(MD, 128).T
        pw.append((w1g, w1p, w2p, np.ascontiguousarray(bt)))
    return pw


def _choose_capacity(counts):
    """Smallest C (multiple of 8) with sum_e ceil(c_e/C) <= 16 slots."""
    for c in range(32, 513, 8):
        if sum(-(-n // c) for n in counts if n) <= NSLOT * E:
            return c
    return 512


def kernel(**inputs):
    global LAST_RESULT
    f32 = lambda a: np.ascontiguousarray(np.asarray(a, dtype=np.float32))
    xr = f32(inputs["x_real"]).reshape(NT, D)
    xi = f32(inputs["x_imag"]).reshape(NT, D)
    gW = f32(inputs["gate_W"])
    gb = f32(inputs["gate_b"])

    # ---- gating on host (float64; reference is fp32 -> argmax margins are
    # ~2.5e-4, far above either rounding level)
    xr64, xi64 = xr.astype(np.float64), xi.astype(np.float64)
    amp = np.sqrt(xr64 * xr64 + xi64 * xi64)
    phase = np.arctan2(xi64, xr64)
    scores = amp @ gW[:D].astype(np.float64) + phase @ gW[D:].astype(np.float64)
    scores += gb.astype(np.float64)
    idx = scores.argmax(-1)
    sm = np.exp(scores - scores.max(-1, keepdims=True))
    w_top = np.take_along_axis(sm, idx[:, None], 1)[:, 0] / sm.sum(-1)

    counts = np.bincount(idx, minlength=E)
    C = _choose_capacity(counts)

    # ---- chunks: (expert, token_idx_array), <= C tokens each, 16 total
    chunks = []
    for e in range(E):
        toks = np.nonzero(idx == e)[0]
        for i in range(0, len(toks), C):
            chunks.append((e, toks[i:i + C]))
    while len(chunks) < NSLOT * E:
        chunks.append((0, np.empty(0, np.int64)))
    assert len(chunks) == NSLOT * E, f"capacity search failed: {counts}"

    if ("nc", C) not in _CACHE:
        _CACHE[("nc", C)] = _build_nc(C)
    nc = _CACHE[("nc", C)]

    wkey = (gW.shape, float(inputs["W1r"][0, 0, 0]), float(inputs["W2i"][-1, -1, -1]))
    if _CACHE.get("wkey") != wkey:
        _CACHE["pw"] = _pack_weights(
            f32(inputs["W1r"]), f32(inputs["W1i"]),
            f32(inputs["W2r"]), f32(inputs["W2i"]),
            f32(inputs["b1r"]), f32(inputs["b1i"]), f32(inputs["mod_b"]),
            f32(inputs["b2r"]), f32(inputs["b2i"]))
        _CACHE["wkey"] = wkey
    pw = _CACHE["pw"]

    bf = mybir.dt.np(BF16)
    in_maps = []
    for c in range(E):
        m = {}
        for s in range(NSLOT):
            e, toks = chunks[NSLOT * c + s]
            w1g, w1p, w2p, bt = pw[e]
            m[f"w1_{s}"] = w1g if s == 0 else w1p
            m[f"w2_{s}"] = w2p
            m[f"bt{s}"] = bt
            xp = np.zeros((128, 2, KD, C), bf)
            n = len(toks)
            if n:
                xp[:, 0, :, :n] = xr[toks].T.reshape(KD, 128, n).transpose(1, 0, 2)
                xp[:, 1, :, :n] = xi[toks].T.reshape(KD, 128, n).transpose(1, 0, 2)
            m[f"xp{s}"] = xp
        in_maps.append(m)

    res = run_bass_kernel_spmd(nc, in_maps, list(range(E)))
    LAST_RESULT = res

    out_r = np.zeros((NT, D), np.float32)
    out_i = np.zeros((NT, D), np.float32)
    for c in range(E):
        for s in range(NSLOT):
            e, toks = chunks[NSLOT * c + s]
            n = len(toks)
            if not n:
                continue
            o = np.asarray(res.results[c][f"out{s}"], np.float32)
            wv = w_top[toks].astype(np.float32)[:, None]
            out_r[toks] = o[:, 0].transpose(1, 0, 2).reshape(D, C)[:, :n].T * wv
            out_i[toks] = o[:, 1].transpose(1, 0, 2).reshape(D, C)[:, :n].T * wv
    return out_r.reshape(B, S, D), out_i.reshape(B, S, D)
